# revision 6
# baseline (speedup 1.0000x reference)
"""Trainium2 Bass kernel for nn_MultiHeadAttention_89429809037632.

Linear attention (softplus feature map) with padding masks:
    q = query @ Wq.T ; k = key @ Wk.T ; v = key @ Wv.T   (per-head split)
    pq = softplus(q) ; pk = softplus(k) * keep(key_mask)
    kv = pk^T v (per head, plus a fused ones-column giving sum(pk))
    out = (pq @ kv) / (pq @ sum(pk)) * keep(query_mask)

Sharding across 8 NeuronCores: data-parallel over N=4 batches x
tensor-parallel over 2 head-groups (8 heads x 128 dims = 1024 output
dims each). Host transposes activations/weights so the contraction
dim (D) lands on the SBUF partition axis; each core runs an identical
SPMD program on its shard, outputs are concatenated on host.

Per-core program (Tile framework):
  Phase A: for each 128-key chunk: project K,V (float32r matmuls,
    stationary = key^T tile), softplus+mask -> pk, copy V into a
    [v | 1] block layout, then 8 per-head fp32 matmuls accumulate
    kv_aug (128x129 per head) in PSUM across all 32 chunks.
  Phase B: for each 512-query chunk x head: project Q (float32r),
    softplus -> pq, then per 128-query subchunk one fp32 matmul
    against kv_aug gives [num | den]; epilogue computes
    num * (keep/den) on DVE and streams to DRAM.
"""

import json
import os
import sys
import types

import numpy as np

for _p in ("/opt/trn_rl_repo",):
    if _p not in sys.path and os.path.isdir(_p):
        sys.path.insert(0, _p)

# ``run_bass_kernel_spmd(trace=True)`` imports antenv.axon_hooks, which not
# every image ships. Provide a stub so the import never crashes (returning
# None simply disables NTFF tracing).
try:
    import antenv.axon_hooks  # noqa: F401
except Exception:
    try:
        import antenv

        _m = types.ModuleType("antenv.axon_hooks")
        _HOOK = [None]

        def _get_hook():
            if _HOOK[0] is None:
                try:
                    from trn_agent_boot.trn_boot import _ntff_profile_via_ctypes

                    _HOOK[0] = _ntff_profile_via_ctypes("/opt/axon/libaxon_pjrt.so")
                except Exception:
                    _HOOK[0] = None
            return _HOOK[0]

        _m.get_axon_ntff_profile_hook = _get_hook
        _m.set_axon_ntff_profile_hook = lambda h: _HOOK.__setitem__(0, h)
        sys.modules["antenv.axon_hooks"] = _m
        antenv.axon_hooks = _m
    except Exception:
        pass

import concourse.bass as bass
import concourse.bass_utils as bu
import concourse.mybir as mybir
import concourse.tile as tile

# ---------------------------------------------------------------------------
# Shim 1: this container's walrus accepts only ONE sync-wait per instruction
# ("Too many sync wait commands"); Tile attaches several. Rewrite the BIR
# JSON so excess waits ride on same-engine NoOps immediately before the
# instruction (engine streams are in-order, so this is equivalent).
# Shim 2: upload_artifacts wants a cloud bucket; keep artifacts local.
# ---------------------------------------------------------------------------
_MAX_WAITS = 1


def _split_multi_waits(bir_bytes: bytes) -> bytes:
    d = json.loads(bir_bytes)
    ctr = 0
    changed = False
    for fn in d.get("functions", []):
        for bb in fn.get("blocks", []):
            out = []
            for inst in bb.get("instructions", []):
                si = inst.get("sync_info")
                waits = (si or {}).get("on_wait") or []
                if len(waits) > _MAX_WAITS:
                    changed = True
                    idx = 0
                    while len(waits) - idx > _MAX_WAITS:
                        chunk = waits[idx : idx + _MAX_WAITS]
                        idx += _MAX_WAITS
                        ctr += 1
                        nop = {
                            "engine": inst["engine"],
                            "ins": [],
                            "outs": [],
                            "name": f"I-wsplit-{ctr}",
                            "opcode": "NoOp",
                            "sync_info": {"on_update": [], "on_wait": chunk},
                        }
                        if "debug" in inst:
                            nop["debug"] = inst["debug"]
                        out.append(nop)
                    si["on_wait"] = waits[idx:]
                out.append(inst)
            bb["instructions"] = out
    return json.dumps(d).encode() if changed else bir_bytes


if not getattr(bass.Bass, "_wait_split_shim", False):
    _orig_to_json = bass.Bass.to_json_bytes

    def _to_json_bytes(self) -> bytes:
        return _split_multi_waits(_orig_to_json(self))

    bass.Bass.to_json_bytes = _to_json_bytes
    bass.Bass._wait_split_shim = True
    bu.upload_artifacts = lambda tmpdir: tmpdir

# ---------------------------------------------------------------------------
# Problem shapes (hardcoded per contract)
# ---------------------------------------------------------------------------
N, L, D = 4, 4096, 2048  # batches, seq len (q and k), model dim
H, P = 16, 128  # heads, head dim
NCORES = 8
HL = H // 2  # heads per core (head-group of 8)
OW = HL * P  # per-core projected width (1024)
DC = D // P  # 16 contraction chunks
LC_A = L // P  # 32 key chunks (phase A)
LC_B = L // 512  # 8 query chunks of 512 (phase B)

F32 = mybir.dt.float32
F32R = mybir.dt.float32r
# The ACT tables in this walrus build ship no softplus; synthesize the
# numerically stable form softplus(x) = max(x,0) + ln(1 + exp(-|x|)) from
# set 6 ("natural_log_exp_and_others": abs/exp/ln in one resident table).
ABS = mybir.ActivationFunctionType.Abs
EXP = mybir.ActivationFunctionType.Exp
LN = mybir.ActivationFunctionType.Ln
MUL = mybir.AluOpType.mult
MAX = mybir.AluOpType.max
ADD = mybir.AluOpType.add

# kv_aug per-head column offsets inside the 3-bank PSUM accumulator:
# 3 heads per 2 KiB bank (129 fp32 columns each, none crossing a bank edge).
_KV_BASE = [(h // 3) * 512 + (h % 3) * 129 for h in range(HL)]

TRACE = False  # set True (e.g. from test.py) to capture NTFF profile
LAST_EXEC_TIME_NS = None

_CACHED_NC = None


def _build_nc() -> bass.Bass:
    nc = bass.Bass()
    qT = nc.dram_tensor("qT", (D, L), F32R, kind="ExternalInput")
    kT = nc.dram_tensor("kT", (D, L), F32R, kind="ExternalInput")
    wq = nc.dram_tensor("wq", (D, OW), F32R, kind="ExternalInput")
    wk = nc.dram_tensor("wk", (D, OW), F32R, kind="ExternalInput")
    wv = nc.dram_tensor("wv", (D, OW), F32R, kind="ExternalInput")
    qm = nc.dram_tensor("qm", (P, LC_A), F32, kind="ExternalInput")
    km = nc.dram_tensor("km", (P, LC_A), F32, kind="ExternalInput")
    out = nc.dram_tensor("out", (L, OW), F32, kind="ExternalOutput")

    with tile.TileContext(nc) as tc:
        with (
            tc.tile_pool(name="misc", bufs=1) as misc,
            tc.tile_pool(name="kvsb", bufs=1) as kvpool,
        ):
            qm_sb = misc.tile([P, LC_A], F32)
            km_sb = misc.tile([P, LC_A], F32)
            nc.sync.dma_start(qm_sb[:], qm[:])
            nc.sync.dma_start(km_sb[:], km[:])
            kv_sb = kvpool.tile([P, HL * 129], F32)

            # ---------------- Phase A: K/V projection + kv accumulation ----
            with (
                tc.tile_pool(name="wkv", bufs=1) as wkvp,
                tc.tile_pool(name="kt", bufs=2) as ktp,
                tc.tile_pool(name="pk", bufs=2) as pkp,
                tc.tile_pool(name="vaug", bufs=2) as vap,
                tc.tile_pool(name="projps", bufs=5, space="PSUM") as pps,
                tc.tile_pool(name="kvps", bufs=1, space="PSUM") as kvps,
            ):
                wk_sb = wkvp.tile([P, DC * OW], F32R, tag="wk")
                wv_sb = wkvp.tile([P, DC * OW], F32R, tag="wv")
                for dc in range(DC):
                    nc.sync.dma_start(
                        wk_sb[:, dc * OW : (dc + 1) * OW],
                        wk[dc * P : (dc + 1) * P, :],
                    )
                    nc.sync.dma_start(
                        wv_sb[:, dc * OW : (dc + 1) * OW],
                        wv[dc * P : (dc + 1) * P, :],
                    )

                kv_ps = kvps.tile([P, 1536], F32)

                for c in range(LC_A):
                    kt_sb = ktp.tile([P, DC * P], F32R, tag="kt")
                    for dc in range(DC):
                        nc.sync.dma_start(
                            kt_sb[:, dc * P : (dc + 1) * P],
                            kT[dc * P : (dc + 1) * P, c * P : (c + 1) * P],
                        )
                    kp0 = pps.tile([P, 512], F32, tag="proj")
                    kp1 = pps.tile([P, 512], F32, tag="proj")
                    vp0 = pps.tile([P, 512], F32, tag="proj")
                    vp1 = pps.tile([P, 512], F32, tag="proj")
                    for dc in range(DC):
                        lhsT = kt_sb[:, dc * P : (dc + 1) * P]
                        st = dict(start=(dc == 0), stop=(dc == DC - 1))
                        nc.tensor.matmul(
                            kp0[:], lhsT, wk_sb[:, dc * OW : dc * OW + 512], **st
                        )
                        nc.tensor.matmul(
                            kp1[:], lhsT, wk_sb[:, dc * OW + 512 : dc * OW + 1024], **st
                        )
                        nc.tensor.matmul(
                            vp0[:], lhsT, wv_sb[:, dc * OW : dc * OW + 512], **st
                        )
                        nc.tensor.matmul(
                            vp1[:], lhsT, wv_sb[:, dc * OW + 512 : dc * OW + 1024], **st
                        )

                    pk_sb = pkp.tile([P, OW], F32, tag="pk")
                    for half, kp in ((0, kp0), (1, kp1)):
                        sa = pkp.tile([P, 512], F32, tag="sa")
                        sb = pkp.tile([P, 512], F32, tag="sb")
                        nc.scalar.activation(sa[:], kp[:], ABS)
                        nc.scalar.activation(sb[:], sa[:], EXP, scale=-1.0)
                        nc.scalar.activation(sa[:], sb[:], LN, bias=1.0)
                        nc.vector.scalar_tensor_tensor(
                            pk_sb[:, half * 512 : (half + 1) * 512],
                            kp[:],
                            0.0,
                            sa[:],
                            MAX,
                            ADD,
                        )
                    nc.vector.tensor_scalar_mul(
                        pk_sb[:], pk_sb[:], km_sb[:, c : c + 1]
                    )

                    va_sb = vap.tile([P, HL * 129], F32, tag="vaug")
                    nc.gpsimd.memset(va_sb[:], 1.0)
                    for h in range(HL):
                        src = vp0 if h < 4 else vp1
                        off = (h % 4) * P
                        nc.vector.tensor_copy(
                            va_sb[:, h * 129 : h * 129 + P],
                            src[:, off : off + P],
                        )

                    # start=True clears has_written for the ENTIRE PSUM bank,
                    # so only the first head in each 3-head bank may use it;
                    # siblings rely on that clear (explicitly ordered after it)
                    # and overwrite-on-first-touch via has_written=0.
                    for h in range(HL):
                        bank_first = h % 3 == 0
                        mm = nc.tensor.matmul(
                            kv_ps[:, _KV_BASE[h] : _KV_BASE[h] + 129],
                            pk_sb[:, h * P : (h + 1) * P],
                            va_sb[:, h * 129 : (h + 1) * 129],
                            start=(c == 0 and bank_first),
                            stop=(c == LC_A - 1),
                            skip_group_check=True,
                        )
                        if c == 0:
                            if bank_first:
                                bank_clear_mm = mm
                            else:
                                tile.add_dep_helper(
                                    mm.ins,
                                    bank_clear_mm.ins,
                                    reason="kv bank has_written clear order",
                                )

                for h in range(HL):
                    nc.vector.tensor_copy(
                        kv_sb[:, h * 129 : (h + 1) * 129],
                        kv_ps[:, _KV_BASE[h] : _KV_BASE[h] + 129],
                    )

            # ---------------- Phase B: Q projection + attention epilogue ---
            with (
                tc.tile_pool(name="wq", bufs=1) as wqp,
                tc.tile_pool(name="qt", bufs=2) as qtp,
                tc.tile_pool(name="pq", bufs=2) as pqp,
                tc.tile_pool(name="sc", bufs=4) as scp,
                tc.tile_pool(name="at", bufs=4) as atp,
                tc.tile_pool(name="qpps", bufs=2, space="PSUM") as qpps,
                tc.tile_pool(name="nmps", bufs=4, space="PSUM") as nmps,
            ):
                wq_sb = wqp.tile([P, DC * OW], F32R)
                for dc in range(DC):
                    nc.sync.dma_start(
                        wq_sb[:, dc * OW : (dc + 1) * OW],
                        wq[dc * P : (dc + 1) * P, :],
                    )

                for lc in range(LC_B):
                    qt_sb = qtp.tile([P, DC * 512], F32R, tag="qt")
                    for dc in range(DC):
                        nc.sync.dma_start(
                            qt_sb[:, dc * 512 : (dc + 1) * 512],
                            qT[dc * P : (dc + 1) * P, lc * 512 : (lc + 1) * 512],
                        )
                    for h in range(HL):
                        qp = qpps.tile([P, 512], F32, tag="qp")
                        for dc in range(DC):
                            nc.tensor.matmul(
                                qp[:],
                                wq_sb[:, dc * OW + h * P : dc * OW + (h + 1) * P],
                                qt_sb[:, dc * 512 : (dc + 1) * 512],
                                start=(dc == 0),
                                stop=(dc == DC - 1),
                            )
                        pq_sb = pqp.tile([P, 512], F32, tag="pq")
                        sa = pqp.tile([P, 512], F32, tag="sa")
                        sb = pqp.tile([P, 512], F32, tag="sb")
                        nc.scalar.activation(sa[:], qp[:], ABS)
                        nc.scalar.activation(sb[:], sa[:], EXP, scale=-1.0)
                        nc.scalar.activation(sa[:], sb[:], LN, bias=1.0)
                        nc.vector.scalar_tensor_tensor(
                            pq_sb[:], qp[:], 0.0, sa[:], MAX, ADD
                        )

                        for j in range(4):
                            nm = nmps.tile([P, 129], F32, tag="nm")
                            nc.tensor.matmul(
                                nm[:],
                                pq_sb[:, j * P : (j + 1) * P],
                                kv_sb[:, h * 129 : (h + 1) * 129],
                                start=True,
                                stop=True,
                            )
                            sc = scp.tile([P, 1], F32, tag="sc")
                            nc.vector.reciprocal(sc[:], nm[:, 128:129])
                            at = atp.tile([P, P], F32, tag="at")
                            col = lc * 4 + j
                            nc.vector.tensor_scalar(
                                at[:],
                                nm[:, 0:P],
                                sc[:, 0:1],
                                qm_sb[:, col : col + 1],
                                MUL,
                                MUL,
                            )
                            nc.sync.dma_start(
                                out[
                                    lc * 512 + j * P : lc * 512 + (j + 1) * P,
                                    h * P : (h + 1) * P,
                                ],
                                at[:],
                            )
    return nc


def _get_nc() -> bass.Bass:
    global _CACHED_NC
    if _CACHED_NC is None:
        _CACHED_NC = _build_nc()
    return _CACHED_NC


def kernel(query, key, Wq, Wk, Wv, query_padding_mask, key_padding_mask):
    global LAST_EXEC_TIME_NS
    query = np.asarray(query, dtype=np.float32)
    key = np.asarray(key, dtype=np.float32)
    Wq = np.asarray(Wq, dtype=np.float32)
    Wk = np.asarray(Wk, dtype=np.float32)
    Wv = np.asarray(Wv, dtype=np.float32)
    qmask = np.asarray(query_padding_mask)
    kmask = np.asarray(key_padding_mask)

    nc = _get_nc()

    in_maps = []
    for c in range(NCORES):
        n, g = c // 2, c % 2
        sl = slice(g * OW, (g + 1) * OW)
        qkeep = (~qmask[n]).astype(np.float32).reshape(LC_A, P).T
        kkeep = (~kmask[n]).astype(np.float32).reshape(LC_A, P).T
        in_maps.append(
            {
                "qT": np.ascontiguousarray(query[n].T),
                "kT": np.ascontiguousarray(key[n].T),
                "wq": np.ascontiguousarray(Wq[sl, :].T),
                "wk": np.ascontiguousarray(Wk[sl, :].T),
                "wv": np.ascontiguousarray(Wv[sl, :].T),
                "qm": np.ascontiguousarray(qkeep),
                "km": np.ascontiguousarray(kkeep),
            }
        )

    res = bu.run_bass_kernel_spmd(
        nc, in_maps, core_ids=list(range(NCORES)), trace=TRACE
    )
    LAST_EXEC_TIME_NS = res.exec_time_ns

    full = np.empty((N, L, D), dtype=np.float32)
    for c in range(NCORES):
        n, g = c // 2, c % 2
        full[n, :, g * OW : (g + 1) * OW] = res.results[c]["out"]
    return full


# revision 9
# speedup vs baseline: 1.0242x; 1.0242x over previous
"""Trainium2 Bass kernel for nn_MultiHeadAttention_89429809037632.

Linear attention (softplus feature map) with padding masks:
    q = query @ Wq.T ; k = key @ Wk.T ; v = key @ Wv.T   (per-head split)
    pq = softplus(q) ; pk = softplus(k) * keep(key_mask)
    kv = pk^T v (per head, plus a fused ones-column giving sum(pk))
    out = (pq @ kv) / (pq @ sum(pk)) * keep(query_mask)

Sharding across 8 NeuronCores: data-parallel over N=4 batches x
tensor-parallel over 2 head-groups (8 heads x 128 dims = 1024 output
dims each). Host transposes activations/weights so the contraction
dim (D) lands on the SBUF partition axis; each core runs an identical
SPMD program on its shard, outputs are concatenated on host.

Per-core program (Tile framework):
  Phase A: for each 128-key chunk: project K,V (float32r matmuls,
    stationary = key^T tile), softplus+mask -> pk, copy V into a
    [v | 1] block layout, then 8 per-head fp32 matmuls accumulate
    kv_aug (128x129 per head) in PSUM across all 32 chunks.
  Phase B: for each 512-query chunk x head: project Q (float32r),
    softplus -> pq, then per 128-query subchunk one fp32 matmul
    against kv_aug gives [num | den]; epilogue computes
    num * (keep/den) on DVE and streams to DRAM.
"""

import json
import os
import sys
import types

import numpy as np

for _p in ("/opt/trn_rl_repo",):
    if _p not in sys.path and os.path.isdir(_p):
        sys.path.insert(0, _p)

# ``run_bass_kernel_spmd(trace=True)`` imports antenv.axon_hooks, which not
# every image ships. Provide a stub so the import never crashes (returning
# None simply disables NTFF tracing).
try:
    import antenv.axon_hooks  # noqa: F401
except Exception:
    try:
        import antenv

        _m = types.ModuleType("antenv.axon_hooks")
        _HOOK = [None]

        def _get_hook():
            if _HOOK[0] is None:
                try:
                    from trn_agent_boot.trn_boot import _ntff_profile_via_ctypes

                    _HOOK[0] = _ntff_profile_via_ctypes("/opt/axon/libaxon_pjrt.so")
                except Exception:
                    _HOOK[0] = None
            return _HOOK[0]

        _m.get_axon_ntff_profile_hook = _get_hook
        _m.set_axon_ntff_profile_hook = lambda h: _HOOK.__setitem__(0, h)
        sys.modules["antenv.axon_hooks"] = _m
        antenv.axon_hooks = _m
    except Exception:
        pass

import concourse.bass as bass
import concourse.bass_utils as bu
import concourse.mybir as mybir
import concourse.tile as tile

# ---------------------------------------------------------------------------
# Shim 1: this container's walrus accepts only ONE sync-wait per instruction
# ("Too many sync wait commands"); Tile attaches several. Rewrite the BIR
# JSON so excess waits ride on same-engine NoOps immediately before the
# instruction (engine streams are in-order, so this is equivalent).
# Shim 2: upload_artifacts wants a cloud bucket; keep artifacts local.
# ---------------------------------------------------------------------------
_MAX_WAITS = 1


def _split_multi_waits(bir_bytes: bytes) -> bytes:
    d = json.loads(bir_bytes)
    ctr = 0
    changed = False
    for fn in d.get("functions", []):
        for bb in fn.get("blocks", []):
            out = []
            for inst in bb.get("instructions", []):
                si = inst.get("sync_info")
                waits = (si or {}).get("on_wait") or []
                if len(waits) > _MAX_WAITS:
                    changed = True
                    idx = 0
                    while len(waits) - idx > _MAX_WAITS:
                        chunk = waits[idx : idx + _MAX_WAITS]
                        idx += _MAX_WAITS
                        ctr += 1
                        nop = {
                            "engine": inst["engine"],
                            "ins": [],
                            "outs": [],
                            "name": f"I-wsplit-{ctr}",
                            "opcode": "NoOp",
                            "sync_info": {"on_update": [], "on_wait": chunk},
                        }
                        if "debug" in inst:
                            nop["debug"] = inst["debug"]
                        out.append(nop)
                    si["on_wait"] = waits[idx:]
                out.append(inst)
            bb["instructions"] = out
    return json.dumps(d).encode() if changed else bir_bytes


if not getattr(bass.Bass, "_wait_split_shim", False):
    _orig_to_json = bass.Bass.to_json_bytes

    def _to_json_bytes(self) -> bytes:
        return _split_multi_waits(_orig_to_json(self))

    bass.Bass.to_json_bytes = _to_json_bytes
    bass.Bass._wait_split_shim = True
    bu.upload_artifacts = lambda tmpdir: tmpdir

# ---------------------------------------------------------------------------
# Problem shapes (hardcoded per contract)
# ---------------------------------------------------------------------------
N, L, D = 4, 4096, 2048  # batches, seq len (q and k), model dim
H, P = 16, 128  # heads, head dim
NCORES = 8
HL = H // 2  # heads per core (head-group of 8)
OW = HL * P  # per-core projected width (1024)
DC = D // P  # 16 contraction chunks
LC_A = L // P  # 32 key chunks (phase A)
LC_B = L // 512  # 8 query chunks of 512 (phase B)

F32 = mybir.dt.float32
F32R = mybir.dt.float32r
# The ACT tables in this walrus build ship no softplus; synthesize the
# numerically stable form softplus(x) = max(x,0) + ln(1 + exp(-|x|)) from
# set 6 ("natural_log_exp_and_others": abs/exp/ln in one resident table).
ABS = mybir.ActivationFunctionType.Abs
EXP = mybir.ActivationFunctionType.Exp
LN = mybir.ActivationFunctionType.Ln
MUL = mybir.AluOpType.mult
MAX = mybir.AluOpType.max
ADD = mybir.AluOpType.add

# kv_aug per-head column offsets inside the 3-bank PSUM accumulator:
# 3 heads per 2 KiB bank (129 fp32 columns each, none crossing a bank edge).
_KV_BASE = [(h // 3) * 512 + (h % 3) * 129 for h in range(HL)]

TRACE = False  # set True (e.g. from test.py) to capture NTFF profile
LAST_EXEC_TIME_NS = None

_CACHED_NC = None


def _build_nc() -> bass.Bass:
    nc = bass.Bass()
    qT = nc.dram_tensor("qT", (D, L), F32R, kind="ExternalInput")
    kT = nc.dram_tensor("kT", (D, L), F32R, kind="ExternalInput")
    wq = nc.dram_tensor("wq", (D, OW), F32R, kind="ExternalInput")
    wk = nc.dram_tensor("wk", (D, OW), F32R, kind="ExternalInput")
    wv = nc.dram_tensor("wv", (D, OW), F32R, kind="ExternalInput")
    qm = nc.dram_tensor("qm", (P, LC_A), F32, kind="ExternalInput")
    km = nc.dram_tensor("km", (P, LC_A), F32, kind="ExternalInput")
    out = nc.dram_tensor("out", (L, OW), F32, kind="ExternalOutput")

    with tile.TileContext(nc) as tc:
        with (
            tc.tile_pool(name="misc", bufs=1) as misc,
            tc.tile_pool(name="kvsb", bufs=1) as kvpool,
        ):
            qm_sb = misc.tile([P, LC_A], F32)
            km_sb = misc.tile([P, LC_A], F32)
            nc.sync.dma_start(qm_sb[:], qm[:])
            nc.sync.dma_start(km_sb[:], km[:])
            kv_sb = kvpool.tile([P, HL * 129], F32)

            # ---------------- Phase A: K/V projection + kv accumulation ----
            with (
                tc.tile_pool(name="wkv", bufs=1) as wkvp,
                tc.tile_pool(name="kt", bufs=2) as ktp,
                tc.tile_pool(name="pk", bufs=2) as pkp,
                tc.tile_pool(name="vaug", bufs=2) as vap,
                tc.tile_pool(name="projps", bufs=5, space="PSUM") as pps,
                tc.tile_pool(name="kvps", bufs=1, space="PSUM") as kvps,
            ):
                # Per-d-chunk weight tiles so the first matmuls only wait on
                # their own DMA, not the whole 16 MB preload.
                wk_sb = [wkvp.tile([P, OW], F32R, tag=f"wk{dc}", name=f"wk{dc}") for dc in range(DC)]
                wv_sb = [wkvp.tile([P, OW], F32R, tag=f"wv{dc}", name=f"wv{dc}") for dc in range(DC)]
                for dc in range(DC):
                    nc.sync.dma_start(wk_sb[dc][:], wk[dc * P : (dc + 1) * P, :])
                    nc.sync.dma_start(wv_sb[dc][:], wv[dc * P : (dc + 1) * P, :])

                kv_ps = kvps.tile([P, 1536], F32)

                for c in range(LC_A):
                    kt_sb = [
                        ktp.tile([P, P], F32R, tag=f"kt{dc}", name=f"kt{dc}") for dc in range(DC)
                    ]
                    for dc in range(DC):
                        nc.sync.dma_start(
                            kt_sb[dc][:],
                            kT[dc * P : (dc + 1) * P, c * P : (c + 1) * P],
                        )
                    kp0 = pps.tile([P, 512], F32, tag="proj")
                    kp1 = pps.tile([P, 512], F32, tag="proj")
                    vp0 = pps.tile([P, 512], F32, tag="proj")
                    vp1 = pps.tile([P, 512], F32, tag="proj")
                    for dc in range(DC):
                        lhsT = kt_sb[dc][:]
                        st = dict(start=(dc == 0), stop=(dc == DC - 1))
                        nc.tensor.matmul(
                            kp0[:], lhsT, wk_sb[dc][:, 0:512], **st
                        )
                        nc.tensor.matmul(
                            kp1[:], lhsT, wk_sb[dc][:, 512:1024], **st
                        )
                        nc.tensor.matmul(
                            vp0[:], lhsT, wv_sb[dc][:, 0:512], **st
                        )
                        nc.tensor.matmul(
                            vp1[:], lhsT, wv_sb[dc][:, 512:1024], **st
                        )

                    pk_sb = pkp.tile([P, OW], F32, tag="pk")
                    for half, kp in ((0, kp0), (1, kp1)):
                        sa = pkp.tile([P, 512], F32, tag="sa")
                        sb = pkp.tile([P, 512], F32, tag="sb")
                        nc.scalar.activation(sa[:], kp[:], ABS)
                        nc.scalar.activation(sb[:], sa[:], EXP, scale=-1.0)
                        nc.scalar.activation(sa[:], sb[:], LN, bias=1.0)
                        nc.vector.scalar_tensor_tensor(
                            pk_sb[:, half * 512 : (half + 1) * 512],
                            kp[:],
                            0.0,
                            sa[:],
                            MAX,
                            ADD,
                        )
                    nc.vector.tensor_scalar_mul(
                        pk_sb[:], pk_sb[:], km_sb[:, c : c + 1]
                    )

                    va_sb = vap.tile([P, HL * 129], F32, tag="vaug")
                    nc.gpsimd.memset(va_sb[:], 1.0)
                    for h in range(HL):
                        src = vp0 if h < 4 else vp1
                        off = (h % 4) * P
                        nc.vector.tensor_copy(
                            va_sb[:, h * 129 : h * 129 + P],
                            src[:, off : off + P],
                        )

                    # start=True clears has_written for the ENTIRE PSUM bank,
                    # so only the first head in each 3-head bank may use it;
                    # siblings rely on that clear (explicitly ordered after it)
                    # and overwrite-on-first-touch via has_written=0.
                    for h in range(HL):
                        bank_first = h % 3 == 0
                        mm = nc.tensor.matmul(
                            kv_ps[:, _KV_BASE[h] : _KV_BASE[h] + 129],
                            pk_sb[:, h * P : (h + 1) * P],
                            va_sb[:, h * 129 : (h + 1) * 129],
                            start=(c == 0 and bank_first),
                            stop=(c == LC_A - 1),
                            skip_group_check=True,
                        )
                        if c == 0:
                            if bank_first:
                                bank_clear_mm = mm
                            else:
                                tile.add_dep_helper(
                                    mm.ins,
                                    bank_clear_mm.ins,
                                    reason="kv bank has_written clear order",
                                )

                for h in range(HL):
                    nc.vector.tensor_copy(
                        kv_sb[:, h * 129 : (h + 1) * 129],
                        kv_ps[:, _KV_BASE[h] : _KV_BASE[h] + 129],
                    )

            # ---------------- Phase B: Q projection + attention epilogue ---
            with (
                tc.tile_pool(name="wq", bufs=1) as wqp,
                tc.tile_pool(name="qt", bufs=2) as qtp,
                tc.tile_pool(name="pq", bufs=2) as pqp,
                tc.tile_pool(name="sc", bufs=4) as scp,
                tc.tile_pool(name="at", bufs=4) as atp,
                tc.tile_pool(name="qpps", bufs=2, space="PSUM") as qpps,
                tc.tile_pool(name="nmps", bufs=4, space="PSUM") as nmps,
            ):
                wq_sb = [wqp.tile([P, OW], F32R, tag=f"wq{dc}", name=f"wq{dc}") for dc in range(DC)]
                for dc in range(DC):
                    nc.sync.dma_start(wq_sb[dc][:], wq[dc * P : (dc + 1) * P, :])

                for lc in range(LC_B):
                    qt_sb = [
                        qtp.tile([P, 512], F32R, tag=f"qt{dc}", name=f"qt{dc}") for dc in range(DC)
                    ]
                    for dc in range(DC):
                        nc.sync.dma_start(
                            qt_sb[dc][:],
                            qT[dc * P : (dc + 1) * P, lc * 512 : (lc + 1) * 512],
                        )
                    for h in range(HL):
                        qp = qpps.tile([P, 512], F32, tag="qp")
                        for dc in range(DC):
                            nc.tensor.matmul(
                                qp[:],
                                wq_sb[dc][:, h * P : (h + 1) * P],
                                qt_sb[dc][:],
                                start=(dc == 0),
                                stop=(dc == DC - 1),
                            )
                        pq_sb = pqp.tile([P, 512], F32, tag="pq")
                        sa = pqp.tile([P, 512], F32, tag="sa")
                        sb = pqp.tile([P, 512], F32, tag="sb")
                        nc.scalar.activation(sa[:], qp[:], ABS)
                        nc.scalar.activation(sb[:], sa[:], EXP, scale=-1.0)
                        nc.scalar.activation(sa[:], sb[:], LN, bias=1.0)
                        nc.vector.scalar_tensor_tensor(
                            pq_sb[:], qp[:], 0.0, sa[:], MAX, ADD
                        )

                        for j in range(4):
                            nm = nmps.tile([P, 129], F32, tag="nm")
                            nc.tensor.matmul(
                                nm[:],
                                pq_sb[:, j * P : (j + 1) * P],
                                kv_sb[:, h * 129 : (h + 1) * 129],
                                start=True,
                                stop=True,
                            )
                            sc = scp.tile([P, 1], F32, tag="sc")
                            nc.vector.reciprocal(sc[:], nm[:, 128:129])
                            at = atp.tile([P, P], F32, tag="at")
                            col = lc * 4 + j
                            nc.vector.tensor_scalar(
                                at[:],
                                nm[:, 0:P],
                                sc[:, 0:1],
                                qm_sb[:, col : col + 1],
                                MUL,
                                MUL,
                            )
                            nc.sync.dma_start(
                                out[
                                    lc * 512 + j * P : lc * 512 + (j + 1) * P,
                                    h * P : (h + 1) * P,
                                ],
                                at[:],
                            )
    return nc


def _get_nc() -> bass.Bass:
    global _CACHED_NC
    if _CACHED_NC is None:
        _CACHED_NC = _build_nc()
    return _CACHED_NC


def kernel(query, key, Wq, Wk, Wv, query_padding_mask, key_padding_mask):
    global LAST_EXEC_TIME_NS
    query = np.asarray(query, dtype=np.float32)
    key = np.asarray(key, dtype=np.float32)
    Wq = np.asarray(Wq, dtype=np.float32)
    Wk = np.asarray(Wk, dtype=np.float32)
    Wv = np.asarray(Wv, dtype=np.float32)
    qmask = np.asarray(query_padding_mask)
    kmask = np.asarray(key_padding_mask)

    nc = _get_nc()

    in_maps = []
    for c in range(NCORES):
        n, g = c // 2, c % 2
        sl = slice(g * OW, (g + 1) * OW)
        qkeep = (~qmask[n]).astype(np.float32).reshape(LC_A, P).T
        kkeep = (~kmask[n]).astype(np.float32).reshape(LC_A, P).T
        in_maps.append(
            {
                "qT": np.ascontiguousarray(query[n].T),
                "kT": np.ascontiguousarray(key[n].T),
                "wq": np.ascontiguousarray(Wq[sl, :].T),
                "wk": np.ascontiguousarray(Wk[sl, :].T),
                "wv": np.ascontiguousarray(Wv[sl, :].T),
                "qm": np.ascontiguousarray(qkeep),
                "km": np.ascontiguousarray(kkeep),
            }
        )

    res = bu.run_bass_kernel_spmd(
        nc, in_maps, core_ids=list(range(NCORES)), trace=TRACE
    )
    LAST_EXEC_TIME_NS = res.exec_time_ns

    full = np.empty((N, L, D), dtype=np.float32)
    for c in range(NCORES):
        n, g = c // 2, c % 2
        full[n, :, g * OW : (g + 1) * OW] = res.results[c]["out"]
    return full
